# revision 16
# baseline (speedup 1.0000x reference)
"""GCN 2-layer encoder on 8 Trainium2 NeuronCores (Bass/Tile).

v3: degree-balanced per-core dst->window permutation (host unpermutes the
output) + 64-slot cell granularity (boundary chunks issue two partition-slice
matmuls) cut gather slot padding from ~35% to ~7%. dma_gather fetches 256B
rows, one call per (window-group, quarter) run on 4 SWDGE queues; PE contracts
128-edge chunks against one-hot*norm selectors (DVE/Pool); dense layers are
interleaved into the L1 aggregation; M=H@W2 is all-gathered (256B rows).
"""
import heapq
import numpy as np

import concourse.bacc as bacc
import concourse.mybir as mybir
from concourse import tile
from concourse.bass_utils import run_bass_kernel_spmd

N_NODES = 100000
IN_DIM, HID_DIM, OUT_DIM = 128, 128, 64
N_CORES = 8
SHARD = N_NODES // N_CORES          # 12500
WIN = 128
N_WIN = (SHARD + WIN - 1) // WIN    # 98
SHARD_PAD = N_WIN * WIN             # 12544
WG = 5                              # windows per gather group
HALF = 64                           # cell slot granularity
MAX_CALL_CHUNKS = 8                 # 1024 descriptors = SWDGE ring cap
ELEM = 128                          # fp16 elems per table row (256B)
Q1 = (N_NODES + 3) // 4             # 25000
Q2 = (SHARD_PAD * N_CORES + 3) // 4  # 25088
DT16 = mybir.dt.float16
DT32 = mybir.dt.float32


def _assign_windows(degc):
    """Greedy least-loaded window assignment (capacity 128 dsts/window).
    Returns (window_of_dst, slot_of_dst)."""
    h = [(0, 0, w) for w in range(N_WIN)]
    heapq.heapify(h)
    wof = np.empty(SHARD, np.int64)
    dnof = np.empty(SHARD, np.int64)
    for dnode in np.argsort(-degc, kind="stable"):
        load, cnt, w = heapq.heappop(h)
        wof[dnode] = w
        dnof[dnode] = cnt
        if cnt + 1 < 128:
            heapq.heappush(h, (load + int(degc[dnode]), cnt + 1, w))
    return wof, dnof


def _build_plan(edge_index):
    src = np.asarray(edge_index[0], dtype=np.int64)
    dst = np.asarray(edge_index[1], dtype=np.int64)
    # degree includes the self-loop (as in the reference)
    deg = np.bincount(dst, minlength=N_NODES).astype(np.float64) + 1.0
    dinv = 1.0 / np.sqrt(deg)
    norm = (dinv[src] * dinv[dst]).astype(np.float32)
    loop_nm = (1.0 / deg).astype(np.float32)   # dinv[i]^2
    owner = dst // SHARD

    n_cells = N_WIN * 4
    n_wg = (N_WIN + WG - 1) // WG
    per_core = []
    counts = np.zeros((N_CORES, n_cells), np.int64)
    perms = []  # per core: padded position of each local dst (win*WIN+slot)
    for c in range(N_CORES):
        m = owner == c
        dstl = dst[m] - c * SHARD
        s = src[m]
        nm = norm[m]
        degc = np.bincount(dstl, minlength=SHARD) + 1
        wof, dnof = _assign_windows(degc)
        perms.append(wof * WIN + dnof)
        win = wof[dstl]
        dnv = dnof[dstl]
        q = s // Q1
        wg = win // WG
        order = np.lexsort((win, q, wg))
        s, nm, win, dnv, q = s[order], nm[order], win[order], dnv[order], q[order]
        cell = win * 4 + q
        counts[c] = np.bincount(cell, minlength=n_cells)
        per_core.append((dnv, s, nm, cell))
    cell_halves = np.maximum(
        np.ceil(counts.max(axis=0) / HALF).astype(np.int64), 1)

    def wins_of(g):
        return list(range(g * WG, min((g + 1) * WG, N_WIN)))

    # layout: runs per (wg, q), each run padded to even halves (chunk aligned)
    cell_half_start = np.zeros(n_cells, np.int64)
    calls = []     # (q, chunk_start, n_chunks)
    wgs = []       # (wins, chunk_lo, chunk_hi, call_ids, pieces, diag_lo)
    acc_h = 0
    cell_order = []
    for g in range(n_wg):
        wins = wins_of(g)
        lo_h = acc_h
        call_ids = []
        for q in range(4):
            start_h = acc_h
            for w in wins:
                cl = w * 4 + q
                cell_order.append(cl)
                cell_half_start[cl] = acc_h
                acc_h += cell_halves[cl]
            if (acc_h - start_h) % 2:
                acc_h += 1  # tail pad half keeps runs chunk-aligned
            c0 = start_h // 2
            n_run = (acc_h - start_h) // 2
            while n_run > 0:
                n_call = min(n_run, MAX_CALL_CHUNKS)
                call_ids.append(len(calls))
                calls.append((q, c0, n_call))
                c0 += n_call
                n_run -= n_call
        diag_lo = acc_h // 2
        acc_h += 2 * len(wins)   # one full chunk per window (diag cell)
        wgs.append([wins, lo_h // 2, acc_h // 2, call_ids, None, diag_lo])
    total_chunks = acc_h // 2
    total_slots = total_chunks * 128
    half_cell_pad = acc_h  # placeholder (diag halves handled separately)

    # half -> cell map (-1 = pad)
    half_cell = np.full(acc_h, -1, np.int64)
    for cl in range(n_cells):
        h0 = cell_half_start[cl]
        half_cell[h0:h0 + cell_halves[cl]] = cl

    # per-wg chunk pieces: (chunk, p0, p1, win)
    for entry in wgs:
        wins, lo, hi, call_ids, _, diag_lo = entry
        pieces = []
        for ck in range(lo, diag_lo):
            h0, h1 = 2 * ck, 2 * ck + 1
            c0, c1 = half_cell[h0], half_cell[h1]
            if c0 == c1:
                if c0 >= 0:
                    pieces.append((ck, 0, 128, c0 // 4))
            else:
                if c0 >= 0:
                    pieces.append((ck, 0, 64, c0 // 4))
                if c1 >= 0:
                    pieces.append((ck, 64, 128, c1 // 4))
        for j, w in enumerate(wins):
            pieces.append((diag_lo + j, 0, 128, w))
        entry[4] = pieces

    perm_all = np.concatenate(perms)          # padded position by global node
    row2_of = SHARD_PAD * (np.arange(N_NODES) // SHARD) + perm_all
    # inverse perms: padded position -> local dst (-1 = unused)
    invperms = []
    for c in range(N_CORES):
        inv = np.full(SHARD_PAD, -1, np.int64)
        inv[perms[c]] = np.arange(SHARD)
        invperms.append(inv)
    idx1 = np.zeros((N_CORES, total_slots), np.int16)
    idx2 = np.zeros((N_CORES, total_slots), np.int16)
    dn = np.zeros((N_CORES, total_slots), np.float32)
    nmarr = np.zeros((N_CORES, total_slots), np.float32)
    for c in range(N_CORES):
        dnv, s, nm, cell = per_core[c]
        cnt = counts[c]
        edge_off = np.zeros(n_cells, np.int64)
        pos = 0
        for cl in cell_order:
            edge_off[cl] = pos
            pos += cnt[cl]
        for cl in range(n_cells):
            n_e = int(cnt[cl])
            s0 = int(cell_half_start[cl]) * HALF
            eo = int(edge_off[cl])
            sl = slice(eo, eo + n_e)
            idx1[c, s0:s0 + n_e] = (s[sl] % Q1).astype(np.int16)
            idx2[c, s0:s0 + n_e] = (row2_of[s[sl]] % Q2).astype(np.int16)
            dn[c, s0:s0 + n_e] = dnv[sl].astype(np.float32)
            nmarr[c, s0:s0 + n_e] = nm[sl]
        # diag chunks: dn = slot iota, nm = 1/deg of the dst at that slot
        iota128 = np.arange(128, dtype=np.float32)
        for (wins, lo, hi, call_ids, pieces, diag_lo) in wgs:
            for j, w in enumerate(wins):
                s0 = (diag_lo + j) * 128
                dn[c, s0:s0 + 128] = iota128
                dloc = invperms[c][w * WIN:(w + 1) * WIN]
                valid = dloc >= 0
                gl = c * SHARD + np.where(valid, dloc, 0)
                nmarr[c, s0:s0 + 128] = np.where(valid, loop_nm[gl], 0.0)

    total_cols = total_slots // 16
    idx1_p = np.zeros((N_CORES, 128, total_cols), np.int16)
    idx2_p = np.zeros((N_CORES, 128, total_cols), np.int16)
    for c in range(N_CORES):
        idx1_p[c] = np.tile(idx1[c].reshape(total_cols, 16).T, (8, 1))
        idx2_p[c] = np.tile(idx2[c].reshape(total_cols, 16).T, (8, 1))
    dn_t = dn.reshape(N_CORES, total_chunks, 128).transpose(0, 2, 1).copy()
    nm_t = nmarr.reshape(N_CORES, total_chunks, 128).transpose(0, 2, 1).copy()

    return dict(
        total_chunks=total_chunks, calls=calls, wgs=wgs,
        idx1=idx1_p, idx2=idx2_p, dn=dn_t, nm=nm_t, perms=perms,
        invperms=invperms,
        max_wg_chunks=max(hi - lo for (_, lo, hi, _, _, _) in wgs),
    )


def _emit_agg_phase(nc, tc, plan, table_d, diag_sb, diag_fstep, idx_d, dn_sb,
                    nm_sb, iota_sb, f_out, out_cb, qsize, tag):
    calls = plan["calls"]
    with (
        tc.tile_pool(name=f"idx{tag}", bufs=3) as idxp,
        tc.tile_pool(name=f"msg{tag}", bufs=2) as msgp,
        tc.tile_pool(name=f"s{tag}", bufs=12) as sp,
        tc.tile_pool(name=f"ps{tag}", bufs=WG, space="PSUM") as psp,
    ):
        qn = 0
        for (wins, lo, hi, call_ids, pieces, diag_lo) in plan["wgs"]:
            nch = diag_lo - lo
            msgs = msgp.tile([128, plan["max_wg_chunks"], ELEM], DT16)
            it = idxp.tile([128, plan["max_wg_chunks"] * 8], mybir.dt.int16)
            nc.sync.dma_start(it[:, :nch * 8], idx_d[:, lo * 8: (lo + nch) * 8])
            for ci in call_ids:
                (q, s, n) = calls[ci]
                nidx = n * 128
                cols = nidx // 16
                nc.gpsimd.dma_gather(
                    msgs[:, s - lo: s - lo + n, :],
                    table_d[q * qsize: min((q + 1) * qsize, table_d.shape[0]), :],
                    it[:, (s - lo) * 8: (s - lo) * 8 + cols],
                    nidx, nidx, ELEM, queue_num=qn % 4)
                qn += 1

            first_of_win, last_of_win = {}, {}
            for i, (ck, p0, p1, w) in enumerate(pieces):
                first_of_win.setdefault(w, i)
                last_of_win[w] = i
            psums = {}
            cur_st = [None, -1]
            for i, (ck, p0, p1, w) in enumerate(pieces):
                if cur_st[1] != ck:
                    st = sp.tile([128, WIN], DT16)
                    nc.vector.tensor_scalar(
                        out=st[:], in0=iota_sb[:],
                        scalar1=dn_sb[:, ck:ck + 1], scalar2=nm_sb[:, ck:ck + 1],
                        op0=mybir.AluOpType.is_equal, op1=mybir.AluOpType.mult)
                    cur_st = [st, ck]
                st = cur_st[0]
                if w not in psums:
                    psums[w] = psp.tile([f_out, WIN], DT32, name=f"psw{tag}",
                                        tag=f"psw{tag}")
                if ck >= diag_lo:
                    lhs = diag_sb[:, w * diag_fstep: w * diag_fstep + f_out]
                else:
                    lhs = msgs[p0:p1, ck - lo, 0:f_out]
                nc.tensor.matmul(
                    psums[w][:], lhsT=lhs, rhs=st[p0:p1, :],
                    start=(i == first_of_win[w]),
                    stop=(i == last_of_win[w]))
            for w in wins:
                out_cb(w, psums[w])


def build_kernel(edge_index, w1, b1, w2, b2, x, reps=1):
    plan = _build_plan(edge_index)

    nc = bacc.Bacc("TRN2", num_devices=N_CORES, num_swdge_queues=4)
    nck = plan["total_chunks"]
    xt_d = nc.dram_tensor("xt", [N_NODES, ELEM], DT16, kind="ExternalInput")
    xown_d = nc.dram_tensor("xown", [128, N_WIN * WIN], DT16, kind="ExternalInput")
    idx1_d = nc.dram_tensor("idx1", [128, nck * 8], mybir.dt.int16, kind="ExternalInput")
    idx2_d = nc.dram_tensor("idx2", [128, nck * 8], mybir.dt.int16, kind="ExternalInput")
    dn_d = nc.dram_tensor("dn", [128, nck], DT32, kind="ExternalInput")
    nm_d = nc.dram_tensor("nm", [128, nck], DT32, kind="ExternalInput")
    w1_d = nc.dram_tensor("w1", [128, HID_DIM], DT16, kind="ExternalInput")
    w2_d = nc.dram_tensor("w2", [128, OUT_DIM], DT16, kind="ExternalInput")
    b1_d = nc.dram_tensor("b1", [128, 1], DT32, kind="ExternalInput")
    b2_d = nc.dram_tensor("b2", [OUT_DIM, 1], DT32, kind="ExternalInput")
    iota_d = nc.dram_tensor("iota", [128, WIN], DT16, kind="ExternalInput")
    out_d = nc.dram_tensor("outT", [OUT_DIM, SHARD_PAD], DT32, kind="ExternalOutput")
    m_local = [nc.dram_tensor(f"m_local{i}", [SHARD_PAD, ELEM], DT16,
                              kind="Internal") for i in range(min(reps, 2))]
    m_full = [nc.dram_tensor(f"m_full{i}", [SHARD_PAD * N_CORES, ELEM], DT16,
                             kind="Internal", addr_space="Shared")
              for i in range(min(reps, 2))]

    with tile.TileContext(nc) as tc:
      with (
          tc.tile_pool(name="persist", bufs=1) as pp,
      ):
        iota_sb = pp.tile([128, WIN], DT16)
        nc.sync.dma_start(iota_sb[:], iota_d[:])
        w1_sb = pp.tile([128, HID_DIM], DT16)
        nc.sync.dma_start(w1_sb[:], w1_d[:])
        w2_sb = pp.tile([128, OUT_DIM], DT16)
        nc.sync.dma_start(w2_sb[:], w2_d[:])
        b1_sb = pp.tile([128, 1], DT32)
        nc.sync.dma_start(b1_sb[:], b1_d[:])
        b2_sb = pp.tile([OUT_DIM, 1], DT32)
        nc.sync.dma_start(b2_sb[:], b2_d[:])
        dn_sb = pp.tile([128, nck], DT32)
        nc.sync.dma_start(dn_sb[:], dn_d[:])
        nm_sb = pp.tile([128, nck], DT32)
        nc.sync.dma_start(nm_sb[:], nm_d[:])
        aggT = pp.tile([128, SHARD_PAD], DT16)
        hT = pp.tile([128, SHARD_PAD], DT16)
        xown_sb = pp.tile([128, N_WIN * WIN], DT16)
        nc.sync.dma_start(xown_sb[:], xown_d[:])
        mown_sb = pp.tile([128, N_WIN * OUT_DIM], DT16)

        for _rep in range(reps):
          ml_d = m_local[_rep % 2]
          mf_d = m_full[_rep % 2]
          with (
              tc.tile_pool(name="mtile", bufs=4) as mp,
              tc.tile_pool(name="dps", bufs=2, space="PSUM") as dps,
          ):
            def l1_out(w, ps, ml_d=ml_d):
                nc.vector.tensor_copy(aggT[:, w * WIN:(w + 1) * WIN], ps[:])
                if (w + 1) % 4 == 0 or w == N_WIN - 1:
                    t0 = (w // 4) * 4 * WIN
                    wdt = (w + 1) * WIN - t0
                    ph = dps.tile([128, 512], DT32, name="dph", tag="dph")
                    nc.tensor.matmul(ph[:, :wdt], lhsT=w1_sb[:],
                                     rhs=aggT[:, t0:t0 + wdt],
                                     start=True, stop=True)
                    nc.scalar.activation(hT[:, t0:t0 + wdt], ph[:, :wdt],
                                         mybir.ActivationFunctionType.Relu,
                                         bias=b1_sb[:, 0:1], scale=1.0)
                    for t in range(t0 // WIN, t0 // WIN + wdt // WIN):
                        pm = dps.tile([128, 512], DT32, name="dph", tag="dph")
                        nc.tensor.matmul(pm[:, :OUT_DIM],
                                         lhsT=hT[:, t * 128:(t + 1) * 128],
                                         rhs=w2_sb[:], start=True, stop=True)
                        msb = mp.tile([128, ELEM], DT16)
                        nc.vector.tensor_copy(msb[:, 0:OUT_DIM], pm[:, :OUT_DIM])
                        nc.vector.memset(msb[:, OUT_DIM:ELEM], 0.0)
                        nc.vector.tensor_copy(
                            mown_sb[:, t * OUT_DIM:(t + 1) * OUT_DIM],
                            pm[:, :OUT_DIM])
                        nc.sync.dma_start(ml_d[t * 128:(t + 1) * 128, :], msb[:])

            _emit_agg_phase(nc, tc, plan, xt_d, xown_sb, WIN, idx1_d, dn_sb,
                            nm_sb, iota_sb, IN_DIM, l1_out, Q1, f"g1r{_rep}")

            # ---- all-gather M ----
            tc.strict_bb_all_engine_barrier()
            nc.gpsimd.collective_compute(
                "AllGather", mybir.AluOpType.bypass,
                replica_groups=[list(range(N_CORES))],
                ins=[ml_d[:]], outs=[mf_d[:]])
            tc.strict_bb_all_engine_barrier()

            # ---- layer 2 (batched out writes: 4 windows per DMA) ----
            with tc.tile_pool(name="ostage", bufs=3) as osp:
                obuf = [None]

                def l2_out(w, ps):
                    if w % 4 == 0:
                        obuf[0] = osp.tile([OUT_DIM, 4 * WIN], DT32, name="ot",
                                           tag="ot")
                    j = w % 4
                    nc.scalar.activation(obuf[0][:, j * WIN:(j + 1) * WIN], ps[:],
                                         mybir.ActivationFunctionType.Identity,
                                         bias=b2_sb[:, 0:1], scale=1.0)
                    if (w + 1) % 4 == 0 or w == N_WIN - 1:
                        t0 = (w // 4) * 4 * WIN
                        wdt = (w + 1) * WIN - t0
                        nc.sync.dma_start(out_d[:, t0:t0 + wdt],
                                          obuf[0][:, :wdt])
                _emit_agg_phase(nc, tc, plan, mf_d, mown_sb, OUT_DIM, idx2_d,
                                dn_sb, nm_sb, iota_sb, OUT_DIM, l2_out, Q2,
                                f"g2r{_rep}")
    nc.compile()

    xt = x.astype(np.float16)
    iota = np.tile(np.arange(WIN, dtype=np.float16), (128, 1))
    in_maps = []
    for c in range(N_CORES):
        inv = plan["invperms"][c]
        xo = np.zeros((SHARD_PAD, ELEM), np.float16)
        valid = inv >= 0
        xo[valid] = xt[c * SHARD + inv[valid]]
        xo2 = xo.reshape(N_WIN, 128, ELEM).transpose(1, 0, 2).reshape(128, -1)
        in_maps.append({
            "xt": xt, "xown": np.ascontiguousarray(xo2),
            "idx1": plan["idx1"][c], "idx2": plan["idx2"][c],
            "dn": plan["dn"][c], "nm": plan["nm"][c],
            "w1": w1.astype(np.float16), "w2": w2.astype(np.float16),
            "b1": b1.reshape(-1, 1).astype(np.float32),
            "b2": b2.reshape(-1, 1).astype(np.float32),
            "iota": iota,
        })
    global _last_plan
    _last_plan = plan
    return nc, in_maps


_last_plan = None


def kernel(x, edge_index, W1, b1, W2, b2):
    x = np.asarray(x); edge_index = np.asarray(edge_index)
    W1 = np.asarray(W1); b1 = np.asarray(b1)
    W2 = np.asarray(W2); b2 = np.asarray(b2)
    nc, in_maps = build_kernel(edge_index, W1, b1, W2, b2, x)
    plan = _last_plan
    res = run_bass_kernel_spmd(nc, in_maps, core_ids=list(range(N_CORES)))
    out = np.empty((N_NODES, OUT_DIM), np.float32)
    for c in range(N_CORES):
        outT = res.results[c]["outT"]          # [OUT_DIM, SHARD_PAD]
        out[c * SHARD:(c + 1) * SHARD] = outT.T[plan["perms"][c]]
    return out
